# revision 19
# baseline (speedup 1.0000x reference)
"""Trainium2 Bass kernel for nn_AttentionRegression (ragged segment attention).

Math reformulation (exact):
  y[b] = g_x*f_x[b] + g_b + num[b]/den[b]
    w_t   = n_t . g_n                     (g weights applied per neighbour row)
    z_t   = exp(sigmoid(tanh(n_t @ W1n^T + f_x[seg]*w1x + b1) @ W2 + b2))
    num_b = sum_{t in seg b} z_t * w_t ;  den_b = sum z_t
  (softmax max-subtraction dropped: scores are sigmoid outputs in (0,1), so
   exp() is stable and the ratio is mathematically unchanged.)

v2: neighbour data ships as fp8 e3m4 (halves HBM traffic vs bf16). Accuracy
is preserved by two host-side tricks that cost the device nothing:
  - bias folding: n' = n + d1*f_x[seg] + d0 with d1 = pinv(W1n) w1x,
    d0 = pinv(W1n) b1, so W1n @ n' = W1n n + w1x fx + b1 exactly and the
    rank-1 bias matmul disappears. The g-path picks up (c1 fx + c0) with
    c1 = g.d1, c0 = g.d0, subtracted per-sample in the epilogue.
  - error-shaped rounding: after round-to-nearest to e3m4, individual
    elements are re-rounded to the adjacent grid point so each row's
    quantized dot with the quantized g-column matches the exact w to ~1e-3.
    This removes the dominant fp8 error path (w feeds the output linearly;
    the tanh/softmax score path is insensitive to 1% noise).

Device layout: segments sorted by length into 16 strata; stratum k supplies one
128-segment block to each of the 8 cores, padded to a common length Ls[k].
Neighbours ship transposed as nt[128 feat, col], col = blockbase + pos*128 +
seg_local, so per-row scalars computed by the PE land as [seg=partition,
pos=free] and segment sums are free-dim reduces. Per 128-row tile the PE does
LDWEIGHTS(nt tile, fp8 fast-weight-load) + one matmul against [128,13]
(12 cols = 16*W1n^T, col 12 = 16*g_n); the 1/32 descale rides the ACT scale.
"""

import numpy as np
import ml_dtypes
from contextlib import ExitStack

import concourse.bass as bass
import concourse.bacc as bacc
import concourse.tile as tile
from concourse import mybir
from concourse.bass_utils import run_bass_kernel_spmd

B, T, NF, H = 16384, 1048576, 128, 12
NCORES = 8
SEGS_PER_BLOCK = 128
CH = 32  # positions per superchunk (psum [128, 13*CH])
ALPHA, BETA = 2.0, 16.0   # fp8 pre-scales: data*ALPHA, weights*BETA
F32 = mybir.dt.float32
BF16 = mybir.dt.bfloat16
F8E3 = mybir.dt.float8e3
AL = mybir.AluOpType
AF = mybir.ActivationFunctionType
E3M4 = ml_dtypes.float8_e3m4

_program_cache = {}


def build_program(Ls, nblk, nrep=1, dual_dma=False, ch=CH, probe="full",
                  dch_mult=4):
    """probe: 'full' | 'dma' (nt streaming only) | 'compute' (no nt streaming,
    matmuls read one preloaded chunk) — for HW bottleneck attribution."""
    nc = bacc.Bacc(
        "TRN2",
        target_bir_lowering=False,
        debug=False,
        enable_asserts=False,
    )
    sumL = sum(Ls)
    R = 128 * sumL
    nt = nc.dram_tensor("nt", [128, R], F8E3, kind="ExternalInput").ap()
    w13 = nc.dram_tensor("w13", [128, 13], F8E3, kind="ExternalInput").ap()
    w2rep = nc.dram_tensor("w2rep", [128, CH * H], BF16, kind="ExternalInput").ap()
    # aux3 cols: 0 b2/2 | 1 gx | 2 gb
    aux3 = nc.dram_tensor("aux3", [128, 3], F32, kind="ExternalInput").ap()
    fxd = nc.dram_tensor("fx", [128, nblk], F32, kind="ExternalInput").ap()
    wcord = nc.dram_tensor("wcor", [128, nblk], F32, kind="ExternalInput").ap()
    maskd = nc.dram_tensor("mask", [128, sumL], BF16, kind="ExternalInput").ap()
    yd = nc.dram_tensor("y", [128, nblk], F32, kind="ExternalOutput").ap()

    with tile.TileContext(nc) as tc, ExitStack() as ctx:
        singles = ctx.enter_context(tc.tile_pool(name="singles", bufs=1))
        bigp = ctx.enter_context(tc.tile_pool(name="bigp", bufs=4))
        psp = ctx.enter_context(tc.tile_pool(name="psp", bufs=4, space="PSUM"))
        hp = ctx.enter_context(tc.tile_pool(name="hp", bufs=3))

        # constants load once, outside the bench loop; small loads ride the
        # gpsimd SWDGE queue so the SP queue can start streaming neighbours
        w13_s = singles.tile([128, 13], F8E3)
        nc.gpsimd.dma_start(out=w13_s[:], in_=w13)
        w2rep_s = singles.tile([128, CH * H], BF16)
        nc.gpsimd.dma_start(out=w2rep_s[:], in_=w2rep)
        aux3_s = singles.tile([128, 3], F32)
        nc.gpsimd.dma_start(out=aux3_s[:], in_=aux3)
        fx_s = singles.tile([128, nblk], F32)
        nc.gpsimd.dma_start(out=fx_s[:], in_=fxd)
        wcor_s = singles.tile([128, nblk], F32)
        nc.gpsimd.dma_start(out=wcor_s[:], in_=wcord)
        mask_s = singles.tile([128, sumL], BF16)
        nc.gpsimd.dma_start(out=mask_s[:], in_=maskd)

        if nrep > 1:
            ctx.enter_context(tc.For_i(0, nrep, 1, name="bench"))

        s_all = singles.tile([128, sumL], F32)
        w_all = singles.tile([128, sumL], F32)
        den_all = singles.tile([128, nblk], F32)
        num_all = singles.tile([128, nblk], F32)

        DESC = 1.0 / (ALPHA * BETA)
        dch = dch_mult * ch  # DMA chunk (positions): bigger transfers, fewer packets
        if probe == "compute":
            ntb_fix = singles.tile([128, 128 * ch], F8E3)
            nc.sync.dma_start(out=ntb_fix[:], in_=nt[:, 0: 128 * ch])
        col = 0
        soff = 0
        nchunk = 0
        for g in range(nblk):
            L = Ls[g]
            ntb_d = None
            for p0 in range(0, L, ch):
                c = min(ch, L - p0)
                if probe == "compute":
                    ntb = ntb_fix
                    nb0 = 0
                else:
                    if p0 % dch == 0:
                        dc = min(dch, L - p0)
                        ntb_d = bigp.tile([128, 128 * dc], F8E3, tag="ntb")
                        eng = nc.gpsimd if (dual_dma and nchunk % 2) else nc.sync
                        nchunk += 1
                        eng.dma_start(
                            out=ntb_d[:],
                            in_=nt[:, col + p0 * 128: col + (p0 + dc) * 128])
                    ntb = ntb_d
                    nb0 = (p0 % dch) * 128
                if probe == "dma":
                    continue
                ps = psp.tile([128, 13 * c], F32, tag="ps")
                for i in range(c):
                    nc.tensor.matmul(
                        ps[:, 13 * i: 13 * (i + 1)],
                        lhsT=ntb[:, nb0 + i * 128: nb0 + (i + 1) * 128],
                        rhs=w13_s[:], start=True, stop=True,
                        skip_group_check=True)
                psv = ps[:].rearrange("p (c t) -> p c t", t=13)
                th = hp.tile([128, c * H], BF16, tag="th")
                nc.scalar.activation(
                    out=th[:].rearrange("p (c t) -> p c t", t=H),
                    in_=psv[:, :, 0:12], func=AF.Tanh, scale=DESC)
                m = hp.tile([128, c * H], BF16, tag="m")
                nc.vector.tensor_mul(m[:], th[:], w2rep_s[:, 0: c * H])
                nc.vector.reduce_sum(
                    out=s_all[:, soff + p0: soff + p0 + c],
                    in_=m[:].rearrange("p (c t) -> p c t", t=H),
                    axis=mybir.AxisListType.X)
                nc.scalar.activation(
                    out=w_all[:, soff + p0: soff + p0 + c],
                    in_=psv[:, :, 12], func=AF.Copy, scale=DESC)

            if probe == "dma":
                col += 128 * L
                soff += L
                continue
            # per-block epilogue, fully inside the {Tanh, Exp, Copy} func set:
            # sigmoid(x) = 0.5 + 0.5*tanh(x/2) and softmax drops constants, so
            # z = exp(0.5*tanh(0.5*(s + b2))) has the exact softmax ratios.
            u = hp.tile([128, L], F32, tag="u")
            nc.scalar.activation(out=u[:], in_=s_all[:, soff: soff + L],
                                 func=AF.Tanh, bias=aux3_s[:, 0:1], scale=0.5)
            z = hp.tile([128, L], F32, tag="z")
            nc.scalar.activation(out=z[:], in_=u[:], func=AF.Exp, scale=0.5)
            zm = hp.tile([128, L], F32, tag="zm")
            nc.vector.tensor_mul(zm[:], z[:], mask_s[:, soff: soff + L])
            # zw = zm * (w_all - (c1*fx + c0))  [wcor holds the negated term]
            zw = hp.tile([128, L], F32, tag="zw")
            nc.vector.scalar_tensor_tensor(
                out=zw[:], in0=w_all[:, soff: soff + L],
                scalar=wcor_s[:, g:g + 1], in1=zm[:],
                op0=AL.add, op1=AL.mult)
            nc.vector.reduce_sum(out=den_all[:, g:g + 1], in_=zm[:],
                                 axis=mybir.AxisListType.X)
            nc.vector.reduce_sum(out=num_all[:, g:g + 1], in_=zw[:],
                                 axis=mybir.AxisListType.X)
            col += 128 * L
            soff += L

        if probe == "dma":
            y_all = singles.tile([128, nblk], F32)
            nc.vector.memset(y_all[:], 0.0)
        else:
            den_eps = singles.tile([128, nblk], F32)
            nc.vector.tensor_scalar(
                out=den_eps[:], in0=den_all[:], scalar1=1e-30, scalar2=None,
                op0=AL.add)
            rec_all = singles.tile([128, nblk], F32)
            nc.vector.reciprocal(out=rec_all[:], in_=den_eps[:])
            t_all = singles.tile([128, nblk], F32)
            nc.vector.tensor_mul(t_all[:], num_all[:], rec_all[:])
            y1_all = singles.tile([128, nblk], F32)
            nc.vector.scalar_tensor_tensor(
                out=y1_all[:], in0=fx_s[:], scalar=aux3_s[:, 1:2], in1=t_all[:],
                op0=AL.mult, op1=AL.add)
            y_all = singles.tile([128, nblk], F32)
            nc.vector.tensor_scalar(
                out=y_all[:], in0=y1_all[:], scalar1=aux3_s[:, 2:3], scalar2=None,
                op0=AL.add)
        nc.sync.dma_start(out=yd, in_=y_all[:])
    nc.compile()
    return nc, R


def _e3m4_luts():
    all_bytes = np.arange(256, dtype=np.uint8)
    vals = all_bytes.view(E3M4).astype(np.float32)
    keep = np.isfinite(vals) & (all_bytes != 0x80)  # single zero entry
    fb = all_bytes[keep][np.argsort(vals[keep], kind="stable")]
    up = all_bytes.copy()
    dn = all_bytes.copy()
    up[fb[:-1]] = fb[1:]
    up[fb[-1]] = fb[-1]
    dn[fb[1:]] = fb[:-1]
    dn[fb[0]] = fb[0]
    up[0x80] = up[0x00]
    dn[0x80] = dn[0x00]
    return vals, up, dn


def _shape_rounding(q8, qg_eff, target, iters=3):
    """Flip individual e3m4 roundings so that q @ qg_eff ~= target per row.

    q8: [N, 128] e3m4 grid values (ALPHA-scaled data), modified in place
    qg_eff: [128] float32, effective per-element weight (qg / ALPHA)
    target: [N] float32 exact per-row dot the device should produce
    """
    vals, up, dn = _e3m4_luts()
    qb = q8.view(np.uint8)
    q = vals[qb]
    E = (q @ qg_eff) - target
    rows = np.arange(len(E))
    for _ in range(iters):
        move_up = (np.sign(E)[:, None] * np.sign(qg_eff)[None, :]) < 0
        nxtb = np.where(move_up, up[qb], dn[qb])
        dE = (vals[nxtb] - q) * qg_eff[None, :]
        newE = E[:, None] + dE
        k = np.argmin(np.abs(newE), axis=1)
        better = np.abs(newE[rows, k]) < np.abs(E)
        rb, kb = rows[better], k[better]
        qb[rb, kb] = nxtb[rb, kb]
        q[rb, kb] = vals[nxtb[rb, kb]]
        E[better] = newE[rb, kb]
    return E


def prep_host(f_x, neighbours, seg_ids, f_W1, f_b1, f_W2, f_b2, g_W, g_b):
    """Shard/pack inputs. Returns (Ls, nblk, in_maps, order)."""
    lens_all = np.bincount(seg_ids, minlength=B).astype(np.int64)
    order = np.argsort(-lens_all, kind="stable")
    nblk = B // (SEGS_PER_BLOCK * NCORES)  # 16
    stratum = SEGS_PER_BLOCK * NCORES  # 1024
    Ls = []
    for k in range(nblk):
        m = int(lens_all[order[k * stratum:(k + 1) * stratum]].max())
        Ls.append(max(1, m))
    sumL = sum(Ls)
    R = 128 * sumL

    row_start = np.zeros(B + 1, np.int64)
    row_start[1:] = np.cumsum(lens_all)

    w1x = f_W1[:, 0].astype(np.float64)
    W1n = f_W1[:, 1:].astype(np.float64)
    gn = g_W[0, 1:].astype(np.float64)
    pinv = np.linalg.pinv(W1n)
    d1 = pinv @ w1x                      # [128]
    d0 = pinv @ f_b1.astype(np.float64)  # [128]
    c1 = float(gn @ d1)
    c0 = float(gn @ d0)

    # fold the per-sample bias into the data, quantize, and error-shape the
    # rounding against the quantized g column
    fx_flat = f_x[seg_ids, 0].astype(np.float32)          # [T]
    nfold = neighbours + np.outer(fx_flat, d1.astype(np.float32))
    nfold += d0.astype(np.float32)[None, :]
    w_true = (nfold @ gn.astype(np.float32)).astype(np.float32)
    q8 = (ALPHA * nfold).astype(E3M4)                     # [T, 128]

    w13 = np.zeros((128, 13), np.float32)
    w13[:, 0:12] = BETA * W1n.T
    w13[:, 12] = BETA * gn
    w13q = w13.astype(E3M4)
    qg = np.asarray(w13q[:, 12], np.float32) / np.float32(BETA)  # device g col
    _shape_rounding(q8, qg / np.float32(ALPHA), w_true)

    w2rep = np.tile(f_W2[0].astype(np.float32), CH)
    w2rep = np.tile(w2rep[None, :], (128, 1)).astype(ml_dtypes.bfloat16)

    aux3 = np.zeros((128, 3), np.float32)
    aux3[:, 0] = 0.5 * f_b2[0]
    aux3[:, 1] = g_W[0, 0]
    aux3[:, 2] = g_b[0]

    in_maps = []
    for c in range(NCORES):
        idx = np.empty(R, np.int64)
        valid = np.empty(R, bool)
        fx_mat = np.empty((128, nblk), np.float32)
        mask = np.empty((128, sumL), ml_dtypes.bfloat16)
        off = 0
        soff = 0
        for g in range(nblk):
            Lg = Ls[g]
            gids = order[g * stratum + 128 * c: g * stratum + 128 * (c + 1)]
            pos = np.arange(Lg)[:, None]
            rows = row_start[gids][None, :] + pos          # [Lg, 128]
            val = pos < lens_all[gids][None, :]
            blockn = Lg * 128
            idx[off:off + blockn] = np.where(val, rows, 0).reshape(-1)
            valid[off:off + blockn] = val.reshape(-1)
            fx_mat[:, g] = f_x[gids, 0]
            mask[:, soff:soff + Lg] = val.T.astype(ml_dtypes.bfloat16)
            off += blockn
            soff += Lg
        nrows = q8[idx]                                    # [R, 128] e3m4
        nrows[~valid] = E3M4(0)
        nt_c = np.ascontiguousarray(nrows.T)               # [128, R]
        wcor = (-(c1 * fx_mat.astype(np.float64) + c0)).astype(np.float32)
        in_maps.append({
            "nt": nt_c, "w13": w13q, "w2rep": w2rep, "aux3": aux3,
            "fx": fx_mat, "wcor": wcor, "mask": mask,
        })
    return Ls, nblk, in_maps, order


def assemble_output(results, order, nblk):
    stratum = SEGS_PER_BLOCK * NCORES
    y_full = np.empty(B, np.float32)
    for c in range(NCORES):
        yc = results[c]["y"]  # [128, nblk]
        for g in range(nblk):
            y_full[order[g * stratum + 128 * c: g * stratum + 128 * (c + 1)]] = yc[:, g]
    return y_full[:, None]


def kernel(**inputs) -> np.ndarray:
    args = {k: np.asarray(v) for k, v in inputs.items()}
    Ls, nblk, in_maps, order = prep_host(
        args["f_x"], args["neighbours"], args["seg_ids"],
        args["f_W1"], args["f_b1"], args["f_W2"], args["f_b2"],
        args["g_W"], args["g_b"])
    key = (tuple(Ls), nblk)
    if key not in _program_cache:
        _program_cache[key] = build_program(Ls, nblk)
    nc, _ = _program_cache[key]
    res = run_bass_kernel_spmd(nc, in_maps, core_ids=list(range(NCORES)))
    return assemble_output(res.results, order, nblk)
